# revision 8
# baseline (speedup 1.0000x reference)
"""GNN message-passing (NodeModel) Trainium2 kernel.

Computation (per reference):
    h   = relu(relu(concat(x[row], ea) @ W0 + b0) @ W1 + b1) @ W2 + b2   [E, 128]
    agg = segment_sum(h, col, N)                                          [N, 128]
    out = relu(relu(concat(x, agg) @ V0 + c0) @ V1 + c1) @ V2 + c2       [N, 128]

Distribution: edges are sorted by destination node; each of the 8 cores owns a
contiguous range of 6250 destination nodes and all edges pointing into it, so
no cross-core reduction is needed.  The host pre-gathers x[row] (and
transposes operands) into streaming layouts; all matmuls run in bf16 with
fp32 PSUM accumulation.

Device pipeline per core (all orientations feature-on-partition "^T" except
where noted):
  L1: h1^T = relu(W0x^T xrow^T + W0e^T ea^T + b0)      (ACT bias+relu)
  L2: h2   = relu(h1^T.T @ W1 + 1 (x) b1)  -- "swap" matmul producing h2 in
      natural [edge, feat] orientation; bias applied by a K=1 ones (x) b1
      matmul into the same PSUM bank.
  Aggregation over W2 pushed past the segment sum:
      u^T[:, n] = sum_{e in seg(n)} h2[e, :]  via matmul with an on-chip
      generated one-hot segment matrix (DVE is_equal against an iota const),
      accumulated per 128-node window in PSUM.
  agg^T = W2^T u^T + b2 (x) deg  (deg outer product via K=1 matmul)
  MLP2 in ^T orientation; final out^T stored fp32.
"""

import os
import numpy as np
import ml_dtypes

import concourse.bass as bass
import concourse.bacc as bacc
import concourse.mybir as mybir
import concourse.tile as tile
from concourse.bass_utils import run_bass_kernel_spmd

BF16 = ml_dtypes.bfloat16

N_NODES = 50000
N_EDGES = 800000
NODE_F = 128
EDGE_F = 64
HID = 128
NCORES = 8
NPC = N_NODES // NCORES  # 6250 nodes per core
WIN = 128                # nodes per aggregation window (PSUM columns)


def _f32(a):
    return np.ascontiguousarray(a, dtype=np.float32)


def _bf(a):
    return np.ascontiguousarray(a, dtype=BF16)


# ---------------------------------------------------------------------------
# Host-side packing
# ---------------------------------------------------------------------------

def _plan_tpw(col_sorted, node_lo_list, npc, nw):
    """Max tiles-per-window over all (core, window)."""
    tpw = 1
    for k, lo in enumerate(node_lo_list):
        sel_lo = np.searchsorted(col_sorted, lo)
        sel_hi = np.searchsorted(col_sorted, lo + npc)
        cols = col_sorted[sel_lo:sel_hi] - lo
        cnt = np.bincount(cols >> 7, minlength=nw)
        tpw = max(tpw, int(-(-cnt.max() // 128)))
    return tpw


def _pack_core(rows, cols, ea_sorted_bf, x_bf, node_lo, nw, tpw):
    """Build per-core device input arrays.

    rows/cols: this core's edges sorted by col (cols in [node_lo, node_lo+nw*128)).
    ea_sorted_bf: [E_k, EDGE_F] bf16 edge features in the same order.
    """
    t_tiles = nw * tpw
    t4 = -(-t_tiles // 8) * 8
    epad = t4 * 128
    nodes_pad = nw * WIN

    win = (cols - node_lo) >> 7                      # window id per edge
    # index within window = position among this window's edges
    win_start = np.searchsorted(win, np.arange(nw))  # first edge idx per window
    j = np.arange(len(cols)) - win_start[win]
    slot = win * (tpw * 128) + j
    assert j.max(initial=0) < tpw * 128

    xrowT = np.zeros((NODE_F, epad), dtype=BF16)
    xrowT[:, slot] = x_bf[rows].T

    ea_full = np.zeros((EDGE_F, epad), dtype=BF16)
    ea_full[:, slot] = ea_sorted_bf.T
    # pack pairs of 512-edge groups vertically: [128, epad//2]
    eaTp = np.ascontiguousarray(
        ea_full.reshape(EDGE_F, epad // 1024, 2, 512)
        .transpose(2, 0, 1, 3)
        .reshape(2 * EDGE_F, epad // 2)
    )

    colloc = np.full((128, t4), -1.0, dtype=BF16)
    local = (cols - node_lo) - (win << 7)
    colloc[slot % 128, slot // 128] = local.astype(BF16)

    deg = np.zeros((1, nodes_pad), dtype=BF16)
    cnt = np.bincount(cols - node_lo, minlength=nodes_pad)
    deg[0, :] = cnt.astype(BF16)

    return dict(xrowT=xrowT, eaT=eaTp, colloc=colloc, degT=deg)


# ---------------------------------------------------------------------------
# Bass program
# ---------------------------------------------------------------------------

def _build_bass(nw, tpw, relu2_split=2):
    """Build the SPMD Bass program. Returns nc."""
    t_tiles = nw * tpw
    t4 = -(-t_tiles // 8) * 8
    epad = t4 * 128
    nodes_pad = nw * WIN
    ngrp = t4 // 4

    dt = mybir.dt
    nc = bacc.Bacc("TRN2", target_bir_lowering=False, debug=False)

    # --- I/O ---
    xrowT_d = nc.dram_tensor("xrowT", [128, epad], dt.bfloat16, kind="ExternalInput")
    eaT_d = nc.dram_tensor("eaT", [128, epad // 2], dt.bfloat16, kind="ExternalInput")
    colloc_d = nc.dram_tensor("colloc", [128, t4], dt.bfloat16, kind="ExternalInput")
    xT_d = nc.dram_tensor("xT", [128, nodes_pad], dt.bfloat16, kind="ExternalInput")
    degT_d = nc.dram_tensor("degT", [1, nodes_pad], dt.bfloat16, kind="ExternalInput")
    wnames = ["W0x", "W0e2", "W1", "W2", "V0x", "V0a", "V1", "V2"]
    w_d = {n: nc.dram_tensor(n, [128, 128], dt.bfloat16, kind="ExternalInput")
           for n in wnames}
    b0_d = nc.dram_tensor("b0f", [128, 1], dt.float32, kind="ExternalInput")
    b1rep_d = nc.dram_tensor("b1rep", [1, 512], dt.bfloat16, kind="ExternalInput")
    b2row_d = nc.dram_tensor("b2row", [1, 128], dt.bfloat16, kind="ExternalInput")
    c0_d = nc.dram_tensor("c0f", [128, 1], dt.float32, kind="ExternalInput")
    c1_d = nc.dram_tensor("c1f", [128, 1], dt.float32, kind="ExternalInput")
    c2_d = nc.dram_tensor("c2f", [128, 1], dt.float32, kind="ExternalInput")
    ones_d = nc.dram_tensor("ones1", [1, 128], dt.bfloat16, kind="ExternalInput")
    iota_d = nc.dram_tensor("iota512", [128, 512], dt.bfloat16, kind="ExternalInput")
    outT_d = nc.dram_tensor("outT", [128, nodes_pad], dt.float32, kind="ExternalOutput")

    with tile.TileContext(nc) as tc:
        with (
            tc.tile_pool(name="const", bufs=1) as cpool,
            tc.tile_pool(name="xr", bufs=4) as xr_pool,
            tc.tile_pool(name="ea", bufs=3) as ea_pool,
            tc.tile_pool(name="h1", bufs=3) as h1_pool,
            tc.tile_pool(name="h2n", bufs=3) as h2n_pool,
            tc.tile_pool(name="seg", bufs=3) as seg_pool,
            tc.tile_pool(name="obuf", bufs=2) as o_pool,
            tc.tile_pool(name="ph1", bufs=2, space="PSUM") as ph1_pool,
            tc.tile_pool(name="ph2", bufs=2, space="PSUM") as ph2_pool,
            tc.tile_pool(name="pu", bufs=2, space="PSUM") as pu_pool,
            tc.tile_pool(name="pm", bufs=2, space="PSUM") as pm_pool,
        ):
            # --- persistent tiles ---
            def load_const(dram, shape, dtype, cname):
                t = cpool.tile(shape, dtype, name=cname, tag=cname)
                nc.sync.dma_start(out=t[:], in_=dram.ap())
                return t

            w_t = {n: load_const(w_d[n], [128, 128], dt.bfloat16, f"c_{n}")
                   for n in wnames}
            b0_t = load_const(b0_d, [128, 1], dt.float32, "c_b0")
            b1rep_t = load_const(b1rep_d, [1, 512], dt.bfloat16, "c_b1")
            b2row_t = load_const(b2row_d, [1, 128], dt.bfloat16, "c_b2")
            c0_t = load_const(c0_d, [128, 1], dt.float32, "c_c0")
            c1_t = load_const(c1_d, [128, 1], dt.float32, "c_c1")
            c2_t = load_const(c2_d, [128, 1], dt.float32, "c_c2")
            ones_t = load_const(ones_d, [1, 128], dt.bfloat16, "c_ones")
            iota_t = load_const(iota_d, [128, 512], dt.bfloat16, "c_iota")
            colloc_t = load_const(colloc_d, [128, t4], dt.bfloat16, "c_colloc")
            xT_t = load_const(xT_d, [128, nodes_pad], dt.bfloat16, "c_xT")
            degT_t = load_const(degT_d, [1, nodes_pad], dt.bfloat16, "c_degT")

            uT_t = cpool.tile([128, nodes_pad], dt.bfloat16, name="uT", tag="uT")
            aggT_t = cpool.tile([128, nodes_pad], dt.bfloat16, name="aggT",
                                tag="aggT")
            g1T_t = cpool.tile([128, nodes_pad], dt.bfloat16, name="g1T",
                               tag="g1T")
            g2T_t = cpool.tile([128, nodes_pad], dt.bfloat16, name="g2T",
                               tag="g2T")

            iota_r = iota_t[:].rearrange("p (a b) -> p a b", b=128)

            # ---------------- Phase A: edges ----------------
            pu_tiles = {}
            for g in range(ngrp):
                e0 = g * 512
                xr = xr_pool.tile([128, 512], dt.bfloat16, tag="xr")
                nc.sync.dma_start(out=xr[:], in_=xrowT_d.ap()[:, e0:e0 + 512])
                if g % 2 == 0:
                    ea = ea_pool.tile([128, 512], dt.bfloat16, tag="ea")
                    c0e = (g // 2) * 512
                    nc.sync.dma_start(out=ea[:], in_=eaT_d.ap()[:, c0e:c0e + 512])
                pbase = (g % 2) * 64

                ph1 = ph1_pool.tile([128, 512], dt.float32, tag="ph1")
                nc.tensor.matmul(out=ph1[:], lhsT=w_t["W0x"][:], rhs=xr[:],
                                 start=True, stop=False)
                nc.tensor.matmul(out=ph1[:],
                                 lhsT=w_t["W0e2"][pbase:pbase + 64, :],
                                 rhs=ea[pbase:pbase + 64, :],
                                 start=False, stop=True)
                h1 = h1_pool.tile([128, 512], dt.bfloat16, tag="h1")
                nc.scalar.activation(h1[:], ph1[:],
                                     mybir.ActivationFunctionType.Relu,
                                     bias=b0_t[:])

                ph2 = ph2_pool.tile([128, 512], dt.float32, tag="ph2")
                nc.tensor.matmul(out=ph2[:], lhsT=ones_t[:], rhs=b1rep_t[:],
                                 start=True, stop=False)
                for i in range(4):
                    sl = slice(i * 128, (i + 1) * 128)
                    nc.tensor.matmul(out=ph2[:, sl], lhsT=h1[:, sl],
                                     rhs=w_t["W1"][:],
                                     start=False, stop=(i == 3))
                h2n = h2n_pool.tile([128, 512], dt.bfloat16, tag="h2n")
                if g % relu2_split == 0:
                    nc.vector.tensor_scalar_max(h2n[:], ph2[:], 0.0)
                else:
                    nc.scalar.activation(h2n[:], ph2[:],
                                         mybir.ActivationFunctionType.Relu)

                seg = seg_pool.tile([128, 512], dt.bfloat16, tag="seg")
                cl4 = colloc_t[:, g * 4:(g + 1) * 4].to_broadcast([128, 4, 128])
                nc.vector.tensor_tensor(
                    out=seg[:].rearrange("p (a b) -> p a b", b=128),
                    in0=cl4, in1=iota_r, op=mybir.AluOpType.is_equal)

                for i in range(4):
                    t = g * 4 + i
                    if t >= t_tiles:
                        break
                    w = t // tpw
                    tt = t % tpw
                    sl = slice(i * 128, (i + 1) * 128)
                    if tt == 0:
                        pu_tiles[w] = pu_pool.tile([128, 128], dt.float32,
                                                   name=f"pu{w}", tag="pu")
                    nc.tensor.matmul(out=pu_tiles[w][:], lhsT=h2n[:, sl],
                                     rhs=seg[:, sl],
                                     start=(tt == 0), stop=(tt == tpw - 1))
                    if tt == tpw - 1:
                        nc.vector.tensor_copy(
                            out=uT_t[:, w * 128:(w + 1) * 128],
                            in_=pu_tiles[w][:])
                        del pu_tiles[w]

            # ---------------- Phase B: nodes ----------------
            chunks = []
            c = 0
            while c < nodes_pad:
                n = min(512, nodes_pad - c)
                chunks.append((c, n))
                c += n

            for c, n in chunks:
                sl = slice(c, c + n)
                pagg = pm_pool.tile([128, 512], dt.float32, tag="pm")
                nc.tensor.matmul(out=pagg[:, :n], lhsT=w_t["W2"][:],
                                 rhs=uT_t[:, sl], start=True, stop=False)
                nc.tensor.matmul(out=pagg[:, :n], lhsT=b2row_t[:],
                                 rhs=degT_t[:, sl], start=False, stop=True)
                nc.scalar.activation(aggT_t[:, sl], pagg[:, :n],
                                     mybir.ActivationFunctionType.Copy)
            for c, n in chunks:
                sl = slice(c, c + n)
                pg = pm_pool.tile([128, 512], dt.float32, tag="pm")
                nc.tensor.matmul(out=pg[:, :n], lhsT=w_t["V0x"][:],
                                 rhs=xT_t[:, sl], start=True, stop=False)
                nc.tensor.matmul(out=pg[:, :n], lhsT=w_t["V0a"][:],
                                 rhs=aggT_t[:, sl], start=False, stop=True)
                nc.scalar.activation(g1T_t[:, sl], pg[:, :n],
                                     mybir.ActivationFunctionType.Relu,
                                     bias=c0_t[:])
            for c, n in chunks:
                sl = slice(c, c + n)
                pg = pm_pool.tile([128, 512], dt.float32, tag="pm")
                nc.tensor.matmul(out=pg[:, :n], lhsT=w_t["V1"][:],
                                 rhs=g1T_t[:, sl], start=True, stop=True)
                nc.scalar.activation(g2T_t[:, sl], pg[:, :n],
                                     mybir.ActivationFunctionType.Relu,
                                     bias=c1_t[:])
            for c, n in chunks:
                sl = slice(c, c + n)
                pg = pm_pool.tile([128, 512], dt.float32, tag="pm")
                nc.tensor.matmul(out=pg[:, :n], lhsT=w_t["V2"][:],
                                 rhs=g2T_t[:, sl], start=True, stop=True)
                ob = o_pool.tile([128, 512], dt.float32, tag="ob")
                nc.scalar.activation(ob[:, :n], pg[:, :n],
                                     mybir.ActivationFunctionType.Identity,
                                     bias=c2_t[:])
                nc.sync.dma_start(out=outT_d.ap()[:, sl], in_=ob[:, :n])

    nc.compile()
    return nc


# ---------------------------------------------------------------------------
# Shared-weight input prep
# ---------------------------------------------------------------------------

def _prep_weights(W0, b0, W1, b1, W2, b2, V0, c0, V1, c1, V2, c2):
    W0 = _f32(W0); V0 = _f32(V0)
    w = dict(
        W0x=_bf(W0[:NODE_F]),
        W0e2=_bf(np.vstack([W0[NODE_F:], W0[NODE_F:]])),
        W1=_bf(W1), W2=_bf(W2),
        V0x=_bf(V0[:NODE_F]), V0a=_bf(V0[NODE_F:]),
        V1=_bf(V1), V2=_bf(V2),
        b0f=_f32(b0).reshape(128, 1),
        b1rep=_bf(np.tile(_f32(b1), 4)).reshape(1, 512),
        b2row=_bf(b2).reshape(1, 128),
        c0f=_f32(c0).reshape(128, 1),
        c1f=_f32(c1).reshape(128, 1),
        c2f=_f32(c2).reshape(128, 1),
        ones1=np.ones((1, 128), dtype=BF16),
        iota512=np.ascontiguousarray(
            np.broadcast_to(np.tile(np.arange(128), 4), (128, 512))).astype(BF16),
    )
    return w


# ---------------------------------------------------------------------------
# Entry point
# ---------------------------------------------------------------------------

_LAST_RESULTS = {}


def kernel(x, edge_index, edge_attr, u, batch,
           W0, b0, W1, b1, W2, b2, V0, c0, V1, c1, V2, c2):
    nw = -(-NPC // WIN)  # 49 windows per core
    nodes_pad = nw * WIN

    x_bf = _bf(x)
    ea_f = _f32(edge_attr)
    row = np.asarray(edge_index[0], dtype=np.int64)
    col = np.asarray(edge_index[1], dtype=np.int64)

    order = np.argsort(col, kind="stable")
    row_s, col_s = row[order], col[order]
    ea_s = _bf(ea_f[order])

    node_lo = [k * NPC for k in range(NCORES)]
    tpw = _plan_tpw(col_s, node_lo, NPC, nw) + 1  # +1 margin

    wts = _prep_weights(W0, b0, W1, b1, W2, b2, V0, c0, V1, c1, V2, c2)

    in_maps = []
    for k in range(NCORES):
        lo = node_lo[k]
        a = np.searchsorted(col_s, lo)
        b = np.searchsorted(col_s, lo + NPC)
        core = _pack_core(row_s[a:b], col_s[a:b], ea_s[a:b], x_bf, lo, nw, tpw)
        xT = np.zeros((NODE_F, nodes_pad), dtype=BF16)
        xT[:, :NPC] = x_bf[lo:lo + NPC].T
        core["xT"] = xT
        core.update(wts)
        in_maps.append(core)

    nc = _build_bass(nw, tpw)

    trace = bool(int(os.environ.get("KERNEL_TRACE", "0")))
    kwargs = {}
    if trace:
        kwargs = dict(trace=True, trace_cores=list(range(NCORES)),
                      stitch_traces=False)
    res = run_bass_kernel_spmd(nc, in_maps, core_ids=list(range(NCORES)),
                               **kwargs)
    _LAST_RESULTS["res"] = res

    out = np.empty((N_NODES, NODE_F), dtype=np.float32)
    for k in range(NCORES):
        out[k * NPC:(k + 1) * NPC] = res.results[k]["outT"][:, :NPC].T
    return out
